# revision 1
# baseline (speedup 1.0000x reference)
"""DenseGrid 'closest' embedding lookup on 8 TRN2 NeuronCores.

Strategy (data-parallel over points, codebooks replicated per core):
 - shard the 4M points into 8 chunks of 500K (padded to 507904 = 31
   tiles of 128x128), host-side re-layout to the device tile order;
 - per core, 12 table passes (LOD0-5 whole codebook, LOD6 in 2 chunks,
   LOD7 in 4 chunks of <=16384 rows).  Each pass broadcasts the
   codebook chunk to all 128 SBUF partitions, computes the fp32 cell
   indices on the vector engine exactly as the reference does (exact
   fp32 floor via the 2^23 magic-constant trick), gathers with the
   GPSIMD ap_gather instruction (double-buffered output, extract DMAs
   split across the SP and ACT HWDGE queues), and spills per-pass
   strips to DRAM;
 - the merge (chunk selects for LOD6/7 + interleaving the 8 LODs into
   [N,16] rows) is emitted per-tile inside the last pass so it hides
   under the remaining gathers.  Output rows come back in the original
   point order.
"""
import math
import sys

import numpy as np

for _p in ("/opt/trn_rl_repo", "/root/.axon_site/_ro/trn_rl_repo"):
    if _p not in sys.path:
        sys.path.append(_p)

import concourse.bass as bass
import concourse.tile as tile
from concourse import bacc, mybir
from concourse.bass_utils import run_bass_kernel_spmd

F32 = mybir.dt.float32
I16 = mybir.dt.int16

BASE_RES, MAX_RES, NUM_LOD, FEAT = 16, 256, 8, 2
_growth = math.exp((math.log(MAX_RES) - math.log(BASE_RES)) / (NUM_LOD - 1))
LODS = [int(BASE_RES * _growth ** L) for L in range(NUM_LOD)]   # 16..256
N_PTS = 4_000_000
N_CORES = 8
T = 128                      # points per partition per tile
PTS_PER_TILE = 128 * T       # 16384
N_CORE = N_PTS // N_CORES    # 500000
N_TILES = 32   # y-band shard: per-core capacity 524288
NP_CORE = N_TILES * PTS_PER_TILE                        # 507904


BAND_ROWS = {6: 23, 7: 33}          # proven max rows per y-band
BAND_V = {6: 23 * 172, 7: 33 * 256}  # 3956, 8448

def _make_passes():
    passes, sid = [], 0
    for l, res in enumerate(LODS):
        V = res * res
        if V <= 16384:
            passes.append((l, 0, V, sid)); sid += 1
        else:
            passes.append((l, -1, BAND_V[l], sid)); sid += 1
    return passes


PASSES = _make_passes()
N_STRIPS = len(PASSES)
LOD_STRIPS = {l: [(b, c, s) for (ll, b, c, s) in PASSES if ll == l]
              for l in range(NUM_LOD)}


def _build_kernel(n_tiles=N_TILES):
    nc = bacc.Bacc("TRN2", target_bir_lowering=False, debug=False,
                   num_devices=N_CORES)
    npc = n_tiles * PTS_PER_TILE
    pts = nc.dram_tensor("pts", [128, n_tiles, T, 2], F32, kind="ExternalInput")
    cbs = [nc.dram_tensor(f"cb{i}", [LODS[i] * LODS[i], 2], F32,
                          kind="ExternalInput") for i in range(NUM_LOD)]
    bands = {l: nc.dram_tensor(f"cb{l}band", [BAND_V[l], 2], F32,
                               kind="ExternalInput") for l in (6, 7)}
    strips = [nc.dram_tensor(f"strip{s}", [npc, 2], F32) for s in range(N_STRIPS)]
    out = nc.dram_tensor("out", [npc, 16], F32, kind="ExternalOutput")

    with tile.TileContext(nc) as tc:
        with tc.tile_pool(name="tabp", bufs=1) as tabp, \
             tc.tile_pool(name="gtp", bufs=2) as gtp, \
             tc.tile_pool(name="ptp", bufs=2) as ptp, \
             tc.tile_pool(name="scr", bufs=3) as scr, \
             tc.tile_pool(name="mstr", bufs=1) as mstr, \
             tc.tile_pool(name="mscr", bufs=2) as mscr, \
             tc.tile_pool(name="moutp", bufs=1) as moutp, \
             tc.tile_pool(name="idxp", bufs=2) as idxp:

            def merge_tile(ti, x, y):
                # strips 0..N_STRIPS-1 for tile ti are complete; select chunks
                # (LOD6/7), interleave 8 LODs into [N,16] rows, stream out.
                ot = moutp.tile([128, T, 16], F32, tag="mo")
                stiles = {}
                for (_l, _b, _Vc, _sid) in PASSES:
                    st = mstr.tile([128, T, 2], F32, tag=f"st{_sid}")
                    sap = bass.AP(strips[_sid], ti * PTS_PER_TILE * 2,
                                  [[T * 2, 128], [1, T * 2]])
                    nc.sync.dma_start(st[:], sap)
                    stiles[_sid] = st
                for l2 in range(NUM_LOD):
                    chunks = LOD_STRIPS[l2]
                    if len(chunks) == 1:
                        srctile = stiles[chunks[0][2]]
                    else:
                        res2 = LODS[l2]
                        m2 = float(res2 - 1)
                        MAGIC = 8388608.0
                        xm = mscr.tile([128, T], F32, tag="mxm")
                        fr = mscr.tile([128, T], F32, tag="mfr")
                        fx = mscr.tile([128, T], F32, tag="mfx")
                        idx = mscr.tile([128, T], F32, tag="midx")
                        nc.vector.tensor_scalar_mul(xm[:], x, m2)
                        nc.vector.tensor_scalar(fr[:], xm[:], MAGIC, -MAGIC,
                                                mybir.AluOpType.add,
                                                mybir.AluOpType.add)
                        nc.vector.tensor_tensor(out=fx[:], in0=fr[:], in1=xm[:],
                                                op=mybir.AluOpType.is_gt)
                        nc.vector.tensor_sub(fx[:], fr[:], fx[:])
                        nc.vector.tensor_scalar_mul(xm[:], y, m2)
                        nc.vector.tensor_scalar(fr[:], xm[:], MAGIC, -MAGIC,
                                                mybir.AluOpType.add,
                                                mybir.AluOpType.add)
                        nc.vector.tensor_tensor(out=idx[:], in0=fr[:], in1=xm[:],
                                                op=mybir.AluOpType.is_gt)
                        nc.vector.tensor_sub(xm[:], fr[:], idx[:])
                        nc.vector.scalar_tensor_tensor(
                            out=idx[:], in0=xm[:], scalar=float(res2),
                            in1=fx[:], op0=mybir.AluOpType.mult,
                            op1=mybir.AluOpType.add)
                        cur = stiles[chunks[0][2]]
                        for (b2, Vc2, sid2) in chunks[1:]:
                            mask = mscr.tile([128, T], mybir.dt.uint8, tag="mmask")
                            nc.vector.tensor_scalar(mask[:], idx[:], float(b2),
                                                    None, mybir.AluOpType.is_ge)
                            nxt = mscr.tile([128, T, 2], F32, tag=f"msel{l2}_{sid2}")
                            for f in range(2):
                                nc.vector.select(nxt[:, :, f], mask[:],
                                                 stiles[sid2][:, :, f],
                                                 cur[:, :, f])
                            cur = nxt
                        srctile = cur
                    oap = ot[:]
                    d_ap = bass.AP(oap.tensor, oap.offset + l2,
                                   [[T * 16, 128], [16, T], [8, 2]])
                    nc.vector.tensor_copy(out=d_ap, in_=srctile[:])
                dst = bass.AP(out, ti * PTS_PER_TILE * 16,
                              [[T * 16, 128], [1, T * 16]])
                nc.sync.dma_start(dst, ot[:])

            for pos, (l, base, Vc, sid) in enumerate(PASSES):
                res = LODS[l]
                m = float(res - 1)
                tab = tabp.tile([128, 16384, 2], F32, tag="tab")
                srct = bands[l] if base < 0 else cbs[l]
                src = bass.AP(srct, max(base, 0) * 2, [[0, 128], [2, Vc], [1, 2]])
                nc.sync.dma_start(tab[:, :Vc, :], src)
                for ti in range(n_tiles):
                    pt = ptp.tile([128, T, 2], F32, tag="pt")
                    nc.sync.dma_start(pt[:], pts.ap()[:, ti])
                    x = pt[:, :, 0]
                    y = pt[:, :, 1]
                    xm = scr.tile([128, T], F32, tag="xm")
                    fr = scr.tile([128, T], F32, tag="fr")
                    fx = scr.tile([128, T], F32, tag="fx")
                    idx = scr.tile([128, T], F32, tag="idx")
                    # exact floor via round-to-nearest magic const + fixup
                    MAGIC = 8388608.0
                    nc.vector.tensor_scalar_mul(xm[:], x, m)
                    nc.vector.tensor_scalar(fr[:], xm[:], MAGIC, -MAGIC,
                                            mybir.AluOpType.add,
                                            mybir.AluOpType.add)   # rne(x*m)
                    nc.vector.tensor_tensor(out=fx[:], in0=fr[:], in1=xm[:],
                                            op=mybir.AluOpType.is_gt)
                    nc.vector.tensor_sub(fx[:], fr[:], fx[:])   # floor(x*m)
                    nc.vector.tensor_scalar_mul(xm[:], y, m)
                    nc.vector.tensor_scalar(fr[:], xm[:], MAGIC, -MAGIC,
                                            mybir.AluOpType.add,
                                            mybir.AluOpType.add)
                    nc.vector.tensor_tensor(out=idx[:], in0=fr[:], in1=xm[:],
                                            op=mybir.AluOpType.is_gt)
                    nc.vector.tensor_sub(xm[:], fr[:], idx[:])  # floor(y*m)
                    nc.vector.scalar_tensor_tensor(
                        out=idx[:], in0=xm[:], scalar=float(res),
                        in1=fx[:], op0=mybir.AluOpType.mult,
                        op1=mybir.AluOpType.add)
                    if base < 0:
                        # rs = floor(floor(y*8) * (m/8)); idx -= rs*res
                        nc.vector.tensor_scalar_mul(xm[:], y, 8.0)
                        nc.vector.tensor_scalar(fr[:], xm[:], MAGIC, -MAGIC,
                                                mybir.AluOpType.add,
                                                mybir.AluOpType.add)
                        nc.vector.tensor_tensor(out=fx[:], in0=fr[:], in1=xm[:],
                                                op=mybir.AluOpType.is_gt)
                        nc.vector.tensor_sub(xm[:], fr[:], fx[:])
                        nc.vector.tensor_scalar_mul(xm[:], xm[:], m / 8.0)
                        nc.vector.tensor_scalar(fr[:], xm[:], MAGIC, -MAGIC,
                                                mybir.AluOpType.add,
                                                mybir.AluOpType.add)
                        nc.vector.tensor_tensor(out=fx[:], in0=fr[:], in1=xm[:],
                                                op=mybir.AluOpType.is_gt)
                        nc.vector.tensor_sub(xm[:], fr[:], fx[:])
                        nc.vector.scalar_tensor_tensor(
                            out=idx[:], in0=xm[:], scalar=-float(res),
                            in1=idx[:], op0=mybir.AluOpType.mult,
                            op1=mybir.AluOpType.add)
                    if base != 0 or Vc < res * res:
                        nc.vector.tensor_scalar(idx[:], idx[:], 0.0,
                                                float(Vc - 1),
                                                mybir.AluOpType.max,
                                                mybir.AluOpType.min)
                    idx16 = idxp.tile([128, T], I16, tag="idx16")
                    nc.vector.tensor_copy(out=idx16[:], in_=idx[:])
                    gt = gtp.tile([128, 16 * T, 2], F32, tag="gt")
                    nc.gpsimd.ap_gather(gt[:], tab[:, :Vc, :], idx16[:],
                                        channels=128, num_elems=Vc, d=2,
                                        num_idxs=16 * T)
                    # gather output is replicated across each 16-partition
                    # group; read group g's 16T pairs from partition g*16 and
                    # reorder on the DRAM side: value j -> strip row
                    # g*16T + (j%16)*T + j//16.
                    gap = gt[:]
                    pitch = 16 * T * 2
                    engines = (nc.sync, nc.scalar)
                    for g in range(8):
                        src_ap = bass.AP(gap.tensor,
                                         gap.offset + g * 16 * pitch,
                                         [[pitch, 1], [1, 32 * T]])
                        dst_ap = bass.AP(strips[sid],
                                         ti * PTS_PER_TILE * 2 + g * 16 * T * 2,
                                         [[2, T], [T * 2, 16], [1, 2]])
                        engines[g % 2].dma_start(dst_ap, src_ap)
                    if pos == len(PASSES) - 1:
                        merge_tile(ti, x, y)

    nc.compile()
    return nc


_NC_CACHE = {}


def kernel(pts, cb0, cb1, cb2, cb3, cb4, cb5, cb6, cb7):
    pts = np.ascontiguousarray(np.asarray(pts, dtype=np.float32))
    cbs = [np.ascontiguousarray(np.asarray(c, dtype=np.float32))
           for c in (cb0, cb1, cb2, cb3, cb4, cb5, cb6, cb7)]
    assert pts.shape == (N_PTS, 2)

    if "nc" not in _NC_CACHE:
        _NC_CACHE["nc"] = _build_kernel()
    nc = _NC_CACHE["nc"]

    band = np.minimum(np.floor(pts[:, 1] * np.float32(8)).astype(np.int64), 7)
    order = np.argsort(band, kind="stable")
    counts = np.bincount(band, minlength=8)
    assert counts.max() <= NP_CORE, counts
    spts = pts[order]
    offs = np.concatenate([[0], np.cumsum(counts)])
    in_maps = []
    for c in range(N_CORES):
        seg = spts[offs[c]:offs[c + 1]]
        pad = np.tile(np.array([[0.5, (c + 0.5) / 8.0]], np.float32),
                      (NP_CORE - len(seg), 1))
        p = np.concatenate([seg, pad], 0)
        p = np.ascontiguousarray(
            p.reshape(N_TILES, 128, T, 2).transpose(1, 0, 2, 3))
        m = {"pts": p}
        for i in range(NUM_LOD):
            m[f"cb{i}"] = cbs[i]
        rs7 = int(np.floor(31.875 * c)); rs6 = int(np.floor(21.375 * c))
        m["cb7band"] = np.ascontiguousarray(cbs[7][rs7 * 256: rs7 * 256 + BAND_V[7]])
        m["cb6band"] = np.ascontiguousarray(cbs[6][rs6 * 172: rs6 * 172 + BAND_V[6]])
        in_maps.append(m)

    res = run_bass_kernel_spmd(nc, in_maps, core_ids=list(range(N_CORES)))

    full = np.empty((N_PTS, 16), np.float32)
    full[order] = np.concatenate(
        [res.results[c]["out"][:counts[c]] for c in range(N_CORES)], 0)
    return full



# revision 2
# speedup vs baseline: 11.8049x; 11.8049x over previous
"""DenseGrid 'closest' lookup, v2: block-local tables + DVE selection trees.

Layout: the unit square is cut into a 16x16 grid of blocks; each block's
points (~15.6K of the 4M) pad to one 16384-point tile. Core c owns the 8x4
block rect (x half c&1, y quarter c>>2... c>>1), 32 tiles/core. Because
tile -> block is fixed at compile time, every LOD's reachable cell window
per tile is a compile-time rectangle; the host ships per-tile packed
candidate tables (mtab) and the kernel selects per-point values:

 - tree LODs (small windows): binary selection tree of copy_predicated ops
   on the DVE, keyed by bits of the point's local cell index;
 - gather LODs (big windows): GPSIMD ap_gather from the per-tile window
   table, extracted to DRAM strips with contiguous 1-descriptor DMAs.

Host side: bin points to blocks, pack tiles in (group, wrap16) order,
build mtab, run SPMD on 8 cores, reassemble [N, 16].
"""
import math
import sys

import numpy as np

for _p in ("/opt/trn_rl_repo", "/root/.axon_site/_ro/trn_rl_repo"):
    if _p not in sys.path:
        sys.path.append(_p)

import concourse.bass as bass
import concourse.tile as tile
from concourse import bacc, mybir
from concourse.bass_utils import run_bass_kernel_spmd

F32 = mybir.dt.float32
I32 = mybir.dt.int32
I16 = mybir.dt.int16

BASE_RES, MAX_RES, NUM_LOD, FEAT = 16, 256, 8, 2
_growth = math.exp((math.log(MAX_RES) - math.log(BASE_RES)) / (NUM_LOD - 1))
LODS = [int(BASE_RES * _growth ** L) for L in range(NUM_LOD)]   # 16..256
MS = [r - 1 for r in LODS]
N_PTS = 4_000_000
N_CORES = 8
G = 16                      # block grid per side
T = 128
PTS_PER_TILE = 128 * T      # 16384
N_TILES = 32
NP_CORE = N_TILES * PTS_PER_TILE   # 524288
MAGIC = 8388608.0

TREE_LODS = [0, 1, 2, 3, 4, 5]
GATHER_LODS = [6, 7]

# uniform per-lod window dims (max over blocks)
def _win_dims(m):
    rmax = 0
    for b in range(G):
        r0 = (b * m) // G
        r1 = ((b + 1) * m) // G
        rmax = max(rmax, r1 - r0 + 1)
    return rmax

WIN = [_win_dims(m) for m in MS]          # per-side cells per block window
MWIN = [w * w for w in WIN]               # window cells
P2 = [1 << max(1, (MWIN[l] - 1).bit_length()) for l in range(NUM_LOD)]  # pow2 pad

# mtab layout (per tile, f32 row)
OFF_M, OFF_C, OFF_R0, OFF_C0, OFF_CL = 0, 8, 16, 24, 32
_off = 40
TREE_OFF = {}
for l in TREE_LODS:
    TREE_OFF[l] = _off
    _off += P2[l] * 2
GATH_OFF = {}
for l in GATHER_LODS:
    GATH_OFF[l] = _off
    _off += MWIN[l] * 2
W_MTAB = _off
KOUT = 2 * len(TREE_LODS)


def _core_blocks(c):
    """32 (by, bx) blocks of core c, tile-index order."""
    qy, hx = c >> 1, c & 1
    out = []
    for i in range(4):
        for j in range(8):
            out.append((4 * qy + i, 8 * hx + j))
    return out


def _build_kernel(rep=1, do_tree=True, do_gather=True, act_copy=True, do_masks=True):
    nc = bacc.Bacc("TRN2", target_bir_lowering=False, debug=False,
                   num_devices=N_CORES)
    pts = nc.dram_tensor("pts", [128, N_TILES, T, 2], F32, kind="ExternalInput")
    mtab = nc.dram_tensor("mtab", [N_TILES, W_MTAB], F32, kind="ExternalInput")
    out0 = nc.dram_tensor("out0", [N_TILES * 128, T * KOUT], F32,
                          kind="ExternalOutput")
    strips = {l: nc.dram_tensor(f"strip{l}", [NP_CORE, 2], F32,
                                kind="ExternalOutput") for l in GATHER_LODS}

    with tile.TileContext(nc) as tc:
        with tc.tile_pool(name="mtp", bufs=2) as mtp, \
             tc.tile_pool(name="ptp", bufs=2) as ptp, \
             tc.tile_pool(name="wp", bufs=1) as wp, \
             tc.tile_pool(name="mkp", bufs=1) as mkp, \
             tc.tile_pool(name="vp", bufs=1) as vp, \
             tc.tile_pool(name="op", bufs=2) as op_, \
             tc.tile_pool(name="ip", bufs=2) as ip, \
             tc.tile_pool(name="gp", bufs=2) as gp:

            for _r in range(rep):
                for ti in range(N_TILES):
                    mt = mtp.tile([128, W_MTAB], F32, tag="mt")
                    nc.sync.dma_start(
                        mt[:], bass.AP(mtab, ti * W_MTAB, [[0, 128], [1, W_MTAB]]))
                    pt = ptp.tile([128, T, 2], F32, tag="pt")
                    nc.sync.dma_start(pt[:], pts.ap()[:, ti])

                    def cvec(off):
                        return mt[:, off:off + 8].unsqueeze(1) \
                                 .to_broadcast([128, T, 8])

                    mv, Cv = cvec(OFF_M), cvec(OFF_C)
                    r0v, c0v, clv = cvec(OFF_R0), cvec(OFF_C0), cvec(OFF_CL)

                    A = wp.tile([128, T, 8], F32, tag="A")
                    B = wp.tile([128, T, 8], F32, tag="B")
                    C = wp.tile([128, T, 8], F32, tag="C")
                    FX = wp.tile([128, T, 8], F32, tag="FX")
                    FY = wp.tile([128, T, 8], F32, tag="FY")
                    LOC = wp.tile([128, T, 8], F32, tag="LOC")

                    xb = pt[:, :, 0].unsqueeze(2).to_broadcast([128, T, 8])
                    yb = pt[:, :, 1].unsqueeze(2).to_broadcast([128, T, 8])
                    # fx = floor(x*m)
                    nc.vector.tensor_tensor(out=A[:], in0=xb, in1=mv,
                                            op=mybir.AluOpType.mult)
                    nc.vector.tensor_scalar(B[:], A[:], MAGIC, -MAGIC,
                                            mybir.AluOpType.add,
                                            mybir.AluOpType.add)
                    nc.vector.tensor_tensor(out=C[:], in0=B[:], in1=A[:],
                                            op=mybir.AluOpType.is_gt)
                    nc.vector.tensor_tensor(out=FX[:], in0=B[:], in1=C[:],
                                            op=mybir.AluOpType.subtract)
                    # fy = floor(y*m)
                    nc.vector.tensor_tensor(out=A[:], in0=yb, in1=mv,
                                            op=mybir.AluOpType.mult)
                    nc.vector.tensor_scalar(B[:], A[:], MAGIC, -MAGIC,
                                            mybir.AluOpType.add,
                                            mybir.AluOpType.add)
                    nc.vector.tensor_tensor(out=C[:], in0=B[:], in1=A[:],
                                            op=mybir.AluOpType.is_gt)
                    nc.vector.tensor_tensor(out=FY[:], in0=B[:], in1=C[:],
                                            op=mybir.AluOpType.subtract)
                    # local = (fy-r0)*C + (fx-c0), clamped to [0, cl]
                    nc.vector.tensor_tensor(out=A[:], in0=FY[:], in1=r0v,
                                            op=mybir.AluOpType.subtract)
                    nc.vector.tensor_tensor(out=B[:], in0=FX[:], in1=c0v,
                                            op=mybir.AluOpType.subtract)
                    nc.vector.tensor_tensor(out=C[:], in0=A[:], in1=Cv,
                                            op=mybir.AluOpType.mult)
                    nc.vector.tensor_tensor(out=LOC[:], in0=C[:], in1=B[:],
                                            op=mybir.AluOpType.add)
                    nc.vector.tensor_scalar(LOC[:], LOC[:], 0.0, None,
                                            mybir.AluOpType.max)
                    nc.vector.tensor_tensor(out=LOC[:], in0=LOC[:], in1=clv,
                                            op=mybir.AluOpType.min)

                    # gather lods (emitted before the tree so Pool
                    # overlaps the DVE selection work)
                    for gi, l in enumerate(GATHER_LODS if do_gather else []):
                        M = MWIN[l]
                        tabv = mt[:, GATH_OFF[l]:GATH_OFF[l] + 2 * M] \
                            .rearrange("p (a b) -> p a b", b=2)
                        idx16 = ip.tile([128, T], I16, tag=f"ix{gi}")
                        nc.vector.tensor_copy(out=idx16[:], in_=LOC[:, :, l])
                        gt = gp.tile([128, 2048, 2], F32, tag="gt")
                        nc.gpsimd.ap_gather(gt[:], tabv, idx16[:],
                                            channels=128, num_elems=M,
                                            d=2, num_idxs=2048)
                        gpitch = 2048 * 2
                        for g in range(8):
                            src = bass.AP(gt.tensor,
                                          gt[:].offset + g * 16 * gpitch,
                                          [[gpitch, 1], [1, 4096]])
                            dstp = bass.AP(
                                strips[l],
                                (ti * PTS_PER_TILE + g * 2048) * 2,
                                [[1, 4096]])
                            nc.scalar.dma_start(dstp, src)

                    LI = wp.tile([128, T, 8], I32, tag="LI")
                    nc.vector.tensor_copy(out=LI[:], in_=LOC[:])
                    nlev = max(P2[l] for l in TREE_LODS).bit_length() - 1
                    if not do_masks:
                        nlev = 0
                    mks = []
                    for k in range(nlev):
                        mk = mkp.tile([128, T, 8], I32, tag=f"mk{k}")
                        nc.vector.tensor_scalar(mk[:], LI[:], 1 << k, None,
                                                mybir.AluOpType.bitwise_and)
                        mks.append(mk)

                    ot = op_.tile([128, T, KOUT], F32, tag="ot")
                    V = vp.tile([128, T, max(P2[l] for l in TREE_LODS) // 2, 2],
                                F32, tag="V")
                    for li, l in enumerate(TREE_LODS if do_tree else []):
                        P = P2[l]
                        tb = mt[:, TREE_OFF[l]:TREE_OFF[l] + 2 * P] \
                            .rearrange("p (a b) -> p a b", b=2)
                        S = P // 2
                        ev = tb[:, 0::2, :].unsqueeze(1) \
                            .to_broadcast([128, T, S, 2])
                        od = tb[:, 1::2, :].unsqueeze(1) \
                            .to_broadcast([128, T, S, 2])
                        m0 = mks[0][:, :, l].unsqueeze(2).unsqueeze(3) \
                            .to_broadcast([128, T, S, 2])
                        Vv = V[:, :, :S, :]
                        (nc.scalar.copy(out=Vv, in_=ev) if act_copy else
                         nc.vector.tensor_copy(out=Vv, in_=ev))
                        nc.vector.copy_predicated(Vv, m0, od)
                        # in-place halving: after level k, live candidates sit
                        # at slot stride 2^k; pair (s, s+2^(k-1)) -> s.
                        pstride = V[:].ap[0][0]   # partition stride (elements)
                        tstride = V[:].ap[1][0]   # t stride (elements)
                        base_off = V[:].offset
                        k = 1
                        while S > 1:
                            half = 1 << (k - 1)    # src slot offset
                            ns = S // 2
                            mkl = mks[k][:, :, l].unsqueeze(2).unsqueeze(3) \
                                .to_broadcast([128, T, ns, 2])
                            dstap = bass.AP(V.tensor, base_off,
                                            [[pstride, 128], [tstride, T],
                                             [4 * half, ns], [1, 2]])
                            srcap = bass.AP(V.tensor, base_off + 2 * half,
                                            [[pstride, 128], [tstride, T],
                                             [4 * half, ns], [1, 2]])
                            nc.vector.copy_predicated(dstap, mkl, srcap)
                            S = ns
                            k += 1
                        (nc.scalar.copy(out=ot[:, :, 2 * li:2 * li + 2],
                                        in_=V[:, :, 0, :]) if act_copy else
                         nc.vector.tensor_copy(out=ot[:, :, 2 * li:2 * li + 2],
                                               in_=V[:, :, 0, :]))
                    if not do_tree:
                        nc.vector.tensor_copy(out=ot[:, :, 0:2], in_=pt[:])

                    dst = bass.AP(out0, ti * 128 * T * KOUT,
                                  [[T * KOUT, 128], [1, T * KOUT]])
                    nc.sync.dma_start(dst, ot[:])


    nc.compile()
    return nc


_NC_CACHE = {}


def _prep_core(pts_all, block_of, counts, offs, order, cbs, c):
    """Build in_map for core c. Returns (in_map, src_slots[32*16384] int64)."""
    blocks = _core_blocks(c)
    ptile = np.empty((N_TILES, 128, T, 2), np.float32)
    src = np.full((N_TILES, PTS_PER_TILE), -1, np.int64)
    mtab = np.zeros((N_TILES, W_MTAB), np.float32)
    for ti, (by, bx) in enumerate(blocks):
        b = by * G + bx
        cnt = counts[b]
        idxs = order[offs[b]:offs[b] + cnt]
        seg = pts_all[idxs]
        if cnt < PTS_PER_TILE:
            pad = np.tile(np.array([[(bx + 0.5) / G, (by + 0.5) / G]],
                                   np.float32), (PTS_PER_TILE - cnt, 1))
            seg = np.concatenate([seg, pad], 0)
        src[ti, :cnt] = idxs
        # slot s = g*2048 + t*16 + q  ->  (partition g*16+q, col t)
        ptile[ti] = seg.reshape(8, 128, 16, 2).transpose(0, 2, 1, 3) \
            .reshape(128, T, 2)
        # consts
        row = mtab[ti]
        for l in range(NUM_LOD):
            m = MS[l]
            r0 = (by * m) // G
            c0 = (bx * m) // G
            row[OFF_M + l] = m
            row[OFF_C + l] = WIN[l]
            row[OFF_R0 + l] = r0
            row[OFF_C0 + l] = c0
            row[OFF_CL + l] = MWIN[l] - 1
        for l in range(NUM_LOD):
            m, res = MS[l], LODS[l]
            r0 = (by * m) // G
            c0 = (bx * m) // G
            w = WIN[l]
            ri = np.minimum(r0 + np.arange(w), res - 1)
            ci = np.minimum(c0 + np.arange(w), res - 1)
            vals = cbs[l][(ri[:, None] * res + ci[None, :]).ravel()]  # [w*w,2]
            if l in TREE_OFF:
                padv = np.concatenate(
                    [vals, np.tile(vals[-1:], (P2[l] - MWIN[l], 1))], 0)
                row[TREE_OFF[l]:TREE_OFF[l] + 2 * P2[l]] = padv.ravel()
            else:
                row[GATH_OFF[l]:GATH_OFF[l] + 2 * MWIN[l]] = vals.ravel()
    in_map = {"pts": np.ascontiguousarray(ptile.transpose(1, 0, 2, 3)),
              "mtab": mtab}
    return in_map, src.ravel()


def kernel(pts, cb0, cb1, cb2, cb3, cb4, cb5, cb6, cb7):
    pts = np.ascontiguousarray(np.asarray(pts, dtype=np.float32))
    cbs = [np.ascontiguousarray(np.asarray(cb, dtype=np.float32))
           for cb in (cb0, cb1, cb2, cb3, cb4, cb5, cb6, cb7)]
    assert pts.shape == (N_PTS, 2)

    if "nc" not in _NC_CACHE:
        _NC_CACHE["nc"] = _build_kernel()
    nc = _NC_CACHE["nc"]

    x64 = pts[:, 0].astype(np.float64)
    y64 = pts[:, 1].astype(np.float64)
    bx = np.clip(np.floor(x64 * G).astype(np.int64), 0, G - 1)
    by = np.clip(np.floor(y64 * G).astype(np.int64), 0, G - 1)
    block = by * G + bx
    order = np.argsort(block, kind="stable")
    counts = np.bincount(block, minlength=G * G)
    offs = np.concatenate([[0], np.cumsum(counts)])[:-1]
    overflow_rows = []
    if counts.max() > PTS_PER_TILE:
        for b in np.where(counts > PTS_PER_TILE)[0]:
            extra = order[offs[b] + PTS_PER_TILE: offs[b] + counts[b]]
            overflow_rows.extend(extra.tolist())
            counts[b] = PTS_PER_TILE

    in_maps, srcs = [], []
    for c in range(N_CORES):
        m, s = _prep_core(pts, block, counts, offs, order, cbs, c)
        in_maps.append(m)
        srcs.append(s)

    res = run_bass_kernel_spmd(nc, in_maps, core_ids=list(range(N_CORES)))

    full = np.empty((N_PTS, 16), np.float32)
    for c in range(N_CORES):
        src = srcs[c]
        valid = src >= 0
        sv = src[valid]
        o0 = res.results[c]["out0"].reshape(N_TILES, 8, 16, T, KOUT) \
            .transpose(0, 1, 3, 2, 4).reshape(NP_CORE, KOUT)
        o0 = o0[valid]
        for li, l in enumerate(TREE_LODS):
            full[sv, l] = o0[:, 2 * li]
            full[sv, 8 + l] = o0[:, 2 * li + 1]
        for l in GATHER_LODS:
            st = res.results[c][f"strip{l}"][valid]
            full[sv, l] = st[:, 0]
            full[sv, 8 + l] = st[:, 1]

    if overflow_rows:
        ov = np.array(overflow_rows, np.int64)
        for l in range(NUM_LOD):
            m, res_l = MS[l], LODS[l]
            fx = np.floor(pts[ov, 0] * np.float32(m)).astype(np.int32)
            fy = np.floor(pts[ov, 1] * np.float32(m)).astype(np.int32)
            v = cbs[l][fx + fy * res_l]
            full[ov, l] = v[:, 0]
            full[ov, 8 + l] = v[:, 1]
    return full


# revision 3
# speedup vs baseline: 22.5859x; 1.9133x over previous
"""DenseGrid 'closest' lookup, v3: B blocks per tile + interleaved DVE trees.

Generalizes v2: a GX x GY block grid (GX*GY = 256*B blocks); each 16384-point
tile packs B spatially-adjacent blocks, one per 128//B-partition band, so
every LOD's candidate window per (tile band) stays a small compile-time
rectangle. Tree LODs select per-point values with in-place copy_predicated
binary trees (per-lod V buffers, levels interleaved across lods to hide DVE
dispatch latency); gather LODs use GPSIMD ap_gather on the per-band window
tables, extracted to DRAM strips with single-descriptor DMAs.
"""
import math
import sys

import numpy as np

for _p in ("/opt/trn_rl_repo", "/root/.axon_site/_ro/trn_rl_repo"):
    if _p not in sys.path:
        sys.path.append(_p)

import concourse.bass as bass
import concourse.tile as tile
from concourse import bacc, mybir
from concourse.bass_utils import run_bass_kernel_spmd

F32 = mybir.dt.float32
I32 = mybir.dt.int32
I16 = mybir.dt.int16

BASE_RES, MAX_RES, NUM_LOD, FEAT = 16, 256, 8, 2
_growth = math.exp((math.log(MAX_RES) - math.log(BASE_RES)) / (NUM_LOD - 1))
LODS = [int(BASE_RES * _growth ** L) for L in range(NUM_LOD)]   # 16..256
MS = [r - 1 for r in LODS]
N_PTS = 4_000_000
N_CORES = 8
T = 128
PTS_PER_TILE = 128 * T      # 16384
N_TILES = 32
NP_CORE = N_TILES * PTS_PER_TILE   # 524288
MAGIC = 8388608.0

# ---- configuration ----
GX, GY = 32, 32             # block grid
TREE_LODS = [0, 1, 2, 3, 4, 5, 6]
GATHER_LODS = [7]
# -----------------------

NBLK = GX * GY
B = NBLK // 256             # blocks per tile
PB = 128 // B               # partitions per block band
GB = 8 // B                 # gather groups per block band
CAP = N_PTS and (256 * PTS_PER_TILE) // NBLK   # points per block
assert CAP * NBLK == 256 * PTS_PER_TILE


def _win(m, g):
    w = 0
    for b in range(g):
        w = max(w, ((b + 1) * m) // g - (b * m) // g + 1)
    return w

WX = [_win(m, GX) for m in MS]
WY = [_win(m, GY) for m in MS]
MWIN = [WX[l] * WY[l] for l in range(NUM_LOD)]
P2 = [1 << max(1, (MWIN[l] - 1).bit_length()) for l in range(NUM_LOD)]

OFF_M, OFF_C, OFF_R0, OFF_C0, OFF_CL = 0, 8, 16, 24, 32
_off = 40
TREE_OFF = {}
for l in TREE_LODS:
    TREE_OFF[l] = _off
    _off += P2[l] * 2
GATH_OFF = {}
for l in GATHER_LODS:
    GATH_OFF[l] = _off
    _off += MWIN[l] * 2
W_MTAB = _off
KOUT = 2 * len(TREE_LODS)
NLEV = {l: P2[l].bit_length() - 1 for l in TREE_LODS}
MAXLEV = max(NLEV.values())


def _core_blocks(c):
    """List of N_TILES*[B (by,bx) blocks] for core c."""
    qy, hx = c >> 1, c & 1
    ys, xs = GY // 4, GX // 2          # blocks per core region side
    blocks = [(qy * ys + i, hx * xs + j) for i in range(ys) for j in range(xs)]
    # tiles pack B consecutive blocks (row-major within the core region)
    return [blocks[t * B:(t + 1) * B] for t in range(N_TILES)]


def _build_kernel(rep=1, do_tree=True, do_gather=True):
    nc = bacc.Bacc("TRN2", target_bir_lowering=False, debug=False,
                   num_devices=N_CORES)
    pts = nc.dram_tensor("pts", [128, N_TILES, T, 2], F32, kind="ExternalInput")
    mtab = nc.dram_tensor("mtab", [N_TILES * B, W_MTAB], F32,
                          kind="ExternalInput")
    out0 = nc.dram_tensor("out0", [N_TILES * 128, T * KOUT], F32,
                          kind="ExternalOutput")
    strips = {l: nc.dram_tensor(f"strip{l}", [NP_CORE, 2], F32,
                                kind="ExternalOutput") for l in GATHER_LODS}

    with tile.TileContext(nc) as tc:
        with tc.tile_pool(name="mtp", bufs=2) as mtp, \
             tc.tile_pool(name="ptp", bufs=2) as ptp, \
             tc.tile_pool(name="wp", bufs=1) as wp, \
             tc.tile_pool(name="mkp", bufs=1) as mkp, \
             tc.tile_pool(name="vp", bufs=1) as vp, \
             tc.tile_pool(name="op", bufs=2) as op_, \
             tc.tile_pool(name="ip", bufs=2) as ip, \
             tc.tile_pool(name="gp", bufs=2) as gp:

            for _r in range(rep):
                for ti in range(N_TILES):
                    mt = mtp.tile([128, W_MTAB], F32, tag="mt")
                    mpitch = mt[:].ap[0][0]
                    for bi in range(B):
                        dst = bass.AP(mt.tensor, mt[:].offset + bi * PB * mpitch,
                                      [[mpitch, PB], [1, W_MTAB]])
                        nc.sync.dma_start(
                            dst, bass.AP(mtab, (ti * B + bi) * W_MTAB,
                                         [[0, PB], [1, W_MTAB]]))
                    pt = ptp.tile([128, T, 2], F32, tag="pt")
                    nc.sync.dma_start(pt[:], pts.ap()[:, ti])

                    def cvec(off):
                        return mt[:, off:off + 8].unsqueeze(1) \
                                 .to_broadcast([128, T, 8])

                    mv, Cv = cvec(OFF_M), cvec(OFF_C)
                    r0v, c0v, clv = cvec(OFF_R0), cvec(OFF_C0), cvec(OFF_CL)

                    A = wp.tile([128, T, 8], F32, tag="A")
                    Bt = wp.tile([128, T, 8], F32, tag="B")
                    Ct = wp.tile([128, T, 8], F32, tag="C")
                    FX = wp.tile([128, T, 8], F32, tag="FX")
                    FY = wp.tile([128, T, 8], F32, tag="FY")
                    LOC = wp.tile([128, T, 8], F32, tag="LOC")

                    xb = pt[:, :, 0].unsqueeze(2).to_broadcast([128, T, 8])
                    yb = pt[:, :, 1].unsqueeze(2).to_broadcast([128, T, 8])
                    nc.vector.tensor_tensor(out=A[:], in0=xb, in1=mv,
                                            op=mybir.AluOpType.mult)
                    nc.vector.tensor_scalar(Bt[:], A[:], MAGIC, -MAGIC,
                                            mybir.AluOpType.add,
                                            mybir.AluOpType.add)
                    nc.vector.tensor_tensor(out=Ct[:], in0=Bt[:], in1=A[:],
                                            op=mybir.AluOpType.is_gt)
                    nc.vector.tensor_tensor(out=FX[:], in0=Bt[:], in1=Ct[:],
                                            op=mybir.AluOpType.subtract)
                    nc.vector.tensor_tensor(out=A[:], in0=yb, in1=mv,
                                            op=mybir.AluOpType.mult)
                    nc.vector.tensor_scalar(Bt[:], A[:], MAGIC, -MAGIC,
                                            mybir.AluOpType.add,
                                            mybir.AluOpType.add)
                    nc.vector.tensor_tensor(out=Ct[:], in0=Bt[:], in1=A[:],
                                            op=mybir.AluOpType.is_gt)
                    nc.vector.tensor_tensor(out=FY[:], in0=Bt[:], in1=Ct[:],
                                            op=mybir.AluOpType.subtract)
                    nc.vector.tensor_tensor(out=A[:], in0=FY[:], in1=r0v,
                                            op=mybir.AluOpType.subtract)
                    nc.vector.tensor_tensor(out=Bt[:], in0=FX[:], in1=c0v,
                                            op=mybir.AluOpType.subtract)
                    nc.vector.tensor_tensor(out=Ct[:], in0=A[:], in1=Cv,
                                            op=mybir.AluOpType.mult)
                    nc.vector.tensor_tensor(out=LOC[:], in0=Ct[:], in1=Bt[:],
                                            op=mybir.AluOpType.add)
                    nc.vector.tensor_scalar(LOC[:], LOC[:], 0.0, None,
                                            mybir.AluOpType.max)
                    nc.vector.tensor_tensor(out=LOC[:], in0=LOC[:], in1=clv,
                                            op=mybir.AluOpType.min)

                    # gather lods first so Pool overlaps the DVE tree
                    for gi, l in enumerate(GATHER_LODS if do_gather else []):
                        M = MWIN[l]
                        tabv = mt[:, GATH_OFF[l]:GATH_OFF[l] + 2 * M] \
                            .rearrange("p (a b) -> p a b", b=2)
                        idx16 = ip.tile([128, T], I16, tag=f"ix{gi}")
                        nc.vector.tensor_copy(out=idx16[:], in_=LOC[:, :, l])
                        gt = gp.tile([128, 2048, 2], F32, tag="gt")
                        nc.gpsimd.ap_gather(gt[:], tabv, idx16[:],
                                            channels=128, num_elems=M,
                                            d=2, num_idxs=2048)
                        gpitch = 2048 * 2
                        for g in range(8):
                            src = bass.AP(gt.tensor,
                                          gt[:].offset + g * 16 * gpitch,
                                          [[gpitch, 1], [1, 4096]])
                            dstp = bass.AP(
                                strips[l],
                                (ti * PTS_PER_TILE + g * 2048) * 2,
                                [[1, 4096]])
                            nc.scalar.dma_start(dstp, src)

                    if do_tree:
                        LI = wp.tile([128, T, 8], I32, tag="LI")
                        nc.vector.tensor_copy(out=LI[:], in_=LOC[:])
                        mks = []
                        for k in range(MAXLEV):
                            mk = mkp.tile([128, T, 8], I32, tag=f"mk{k}")
                            nc.vector.tensor_scalar(mk[:], LI[:], 1 << k, None,
                                                    mybir.AluOpType.bitwise_and)
                            mks.append(mk)

                        ot = op_.tile([128, T, KOUT], F32, tag="ot")
                        Vs, Ss = {}, {}
                        # level 0: ACT copies then DVE predications, per lod
                        for l in TREE_LODS:
                            P = P2[l]
                            S = P // 2
                            V = vp.tile([128, T, S, 2], F32, tag=f"V{l}")
                            Vs[l], Ss[l] = V, S
                            tb = mt[:, TREE_OFF[l]:TREE_OFF[l] + 2 * P] \
                                .rearrange("p (a b) -> p a b", b=2)
                            ev = tb[:, 0::2, :].unsqueeze(1) \
                                .to_broadcast([128, T, S, 2])
                            nc.scalar.copy(out=V[:], in_=ev)
                        for l in TREE_LODS:
                            P = P2[l]
                            S = Ss[l]
                            tb = mt[:, TREE_OFF[l]:TREE_OFF[l] + 2 * P] \
                                .rearrange("p (a b) -> p a b", b=2)
                            od = tb[:, 1::2, :].unsqueeze(1) \
                                .to_broadcast([128, T, S, 2])
                            m0 = mks[0][:, :, l].unsqueeze(2).unsqueeze(3) \
                                .to_broadcast([128, T, S, 2])
                            nc.vector.copy_predicated(Vs[l][:], m0, od)
                        # in-place levels, interleaved across lods
                        for k in range(1, MAXLEV):
                            half = 1 << (k - 1)
                            for l in TREE_LODS:
                                if Ss[l] <= 1 or k >= NLEV[l]:
                                    continue
                                V = Vs[l]
                                ns = Ss[l] // 2
                                pstride = V[:].ap[0][0]
                                tstride = V[:].ap[1][0]
                                base_off = V[:].offset
                                mkl = mks[k][:, :, l].unsqueeze(2).unsqueeze(3) \
                                    .to_broadcast([128, T, ns, 2])
                                dstap = bass.AP(V.tensor, base_off,
                                                [[pstride, 128], [tstride, T],
                                                 [4 * half, ns], [1, 2]])
                                srcap = bass.AP(V.tensor, base_off + 2 * half,
                                                [[pstride, 128], [tstride, T],
                                                 [4 * half, ns], [1, 2]])
                                nc.vector.copy_predicated(dstap, mkl, srcap)
                                Ss[l] = ns
                        for li, l in enumerate(TREE_LODS):
                            assert Ss[l] == 1, (l, Ss[l])
                            nc.scalar.copy(out=ot[:, :, 2 * li:2 * li + 2],
                                           in_=Vs[l][:, :, 0, :])
                    else:
                        ot = op_.tile([128, T, KOUT], F32, tag="ot")
                        nc.vector.tensor_copy(out=ot[:, :, 0:2], in_=pt[:])

                    dst = bass.AP(out0, ti * 128 * T * KOUT,
                                  [[T * KOUT, 128], [1, T * KOUT]])
                    nc.sync.dma_start(dst, ot[:])

    nc.compile()
    return nc


_NC_CACHE = {}


def _prep_core(pts_all, counts, offs, order, cbs, c):
    """Build in_map for core c. Returns (in_map, src_slots[NP_CORE] int64)."""
    tiles = _core_blocks(c)
    ptile = np.empty((N_TILES, 128, T, 2), np.float32)
    src = np.full((N_TILES, PTS_PER_TILE), -1, np.int64)
    mtab = np.zeros((N_TILES * B, W_MTAB), np.float32)
    for ti, blks in enumerate(tiles):
        for bi, (by, bx) in enumerate(blks):
            b = by * GX + bx
            cnt = min(counts[b], CAP)
            idxs = order[offs[b]:offs[b] + cnt]
            seg = pts_all[idxs]
            if cnt < CAP:
                pad = np.tile(np.array([[(bx + 0.5) / GX, (by + 0.5) / GY]],
                                       np.float32), (CAP - cnt, 1))
                seg = np.concatenate([seg, pad], 0)
            src[ti, bi * CAP:bi * CAP + cnt] = idxs
            # band slots: s = g*2048 + t*16 + q -> partition bi*PB+g*16+q
            ptile[ti, bi * PB:(bi + 1) * PB] = \
                seg.reshape(GB, T, 16, 2).transpose(0, 2, 1, 3) \
                   .reshape(PB, T, 2)
            row = mtab[ti * B + bi]
            for l in range(NUM_LOD):
                m = MS[l]
                row[OFF_M + l] = m
                row[OFF_C + l] = WX[l]
                row[OFF_R0 + l] = (by * m) // GY
                row[OFF_C0 + l] = (bx * m) // GX
                row[OFF_CL + l] = MWIN[l] - 1
            for l in range(NUM_LOD):
                m, res = MS[l], LODS[l]
                r0 = (by * m) // GY
                c0 = (bx * m) // GX
                ri = np.minimum(r0 + np.arange(WY[l]), res - 1)
                ci = np.minimum(c0 + np.arange(WX[l]), res - 1)
                vals = cbs[l][(ri[:, None] * res + ci[None, :]).ravel()]
                if l in TREE_OFF:
                    padv = np.concatenate(
                        [vals, np.tile(vals[-1:], (P2[l] - MWIN[l], 1))], 0)
                    row[TREE_OFF[l]:TREE_OFF[l] + 2 * P2[l]] = padv.ravel()
                else:
                    row[GATH_OFF[l]:GATH_OFF[l] + 2 * MWIN[l]] = vals.ravel()
    in_map = {"pts": np.ascontiguousarray(ptile.transpose(1, 0, 2, 3)),
              "mtab": mtab}
    return in_map, src.ravel()


def _host_bin(pts):
    x64 = pts[:, 0].astype(np.float64)
    y64 = pts[:, 1].astype(np.float64)
    bx = np.clip(np.floor(x64 * GX).astype(np.int64), 0, GX - 1)
    by = np.clip(np.floor(y64 * GY).astype(np.int64), 0, GY - 1)
    block = by * GX + bx
    order = np.argsort(block, kind="stable")
    counts = np.bincount(block, minlength=NBLK)
    offs = np.concatenate([[0], np.cumsum(counts)])[:-1]
    overflow = []
    for b in np.where(counts > CAP)[0]:
        overflow.extend(order[offs[b] + CAP: offs[b] + counts[b]].tolist())
    return order, counts, offs, overflow


def kernel(pts, cb0, cb1, cb2, cb3, cb4, cb5, cb6, cb7):
    pts = np.ascontiguousarray(np.asarray(pts, dtype=np.float32))
    cbs = [np.ascontiguousarray(np.asarray(cb, dtype=np.float32))
           for cb in (cb0, cb1, cb2, cb3, cb4, cb5, cb6, cb7)]
    assert pts.shape == (N_PTS, 2)

    if "nc" not in _NC_CACHE:
        _NC_CACHE["nc"] = _build_kernel()
    nc = _NC_CACHE["nc"]

    order, counts, offs, overflow_rows = _host_bin(pts)
    in_maps, srcs = [], []
    for c in range(N_CORES):
        m, s = _prep_core(pts, counts, offs, order, cbs, c)
        in_maps.append(m)
        srcs.append(s)

    res = run_bass_kernel_spmd(nc, in_maps, core_ids=list(range(N_CORES)))

    full = np.empty((N_PTS, 16), np.float32)
    for c in range(N_CORES):
        src = srcs[c]
        valid = src >= 0
        sv = src[valid]
        o0 = res.results[c]["out0"].reshape(N_TILES, 8, 16, T, KOUT) \
            .transpose(0, 1, 3, 2, 4).reshape(NP_CORE, KOUT)[valid]
        for li, l in enumerate(TREE_LODS):
            full[sv, l] = o0[:, 2 * li]
            full[sv, 8 + l] = o0[:, 2 * li + 1]
        for l in GATHER_LODS:
            st = res.results[c][f"strip{l}"][valid]
            full[sv, l] = st[:, 0]
            full[sv, 8 + l] = st[:, 1]

    if overflow_rows:
        ov = np.array(overflow_rows, np.int64)
        for l in range(NUM_LOD):
            m, res_l = MS[l], LODS[l]
            fx = np.floor(pts[ov, 0] * np.float32(m)).astype(np.int32)
            fy = np.floor(pts[ov, 1] * np.float32(m)).astype(np.int32)
            v = cbs[l][fx + fy * res_l]
            full[ov, l] = v[:, 0]
            full[ov, 8 + l] = v[:, 1]
    return full


# revision 5
# speedup vs baseline: 22.9837x; 1.0176x over previous
"""DenseGrid 'closest' lookup, v3: B blocks per tile + interleaved DVE trees.

Generalizes v2: a GX x GY block grid (GX*GY = 256*B blocks); each 16384-point
tile packs B spatially-adjacent blocks, one per 128//B-partition band, so
every LOD's candidate window per (tile band) stays a small compile-time
rectangle. Tree LODs select per-point values with in-place copy_predicated
binary trees (per-lod V buffers, levels interleaved across lods to hide DVE
dispatch latency); gather LODs use GPSIMD ap_gather on the per-band window
tables, extracted to DRAM strips with single-descriptor DMAs.
"""
import math
import sys

import numpy as np

for _p in ("/opt/trn_rl_repo", "/root/.axon_site/_ro/trn_rl_repo"):
    if _p not in sys.path:
        sys.path.append(_p)

import concourse.bass as bass
import concourse.tile as tile
from concourse import bacc, mybir
from concourse.bass_utils import run_bass_kernel_spmd

F32 = mybir.dt.float32
I32 = mybir.dt.int32
I16 = mybir.dt.int16

BASE_RES, MAX_RES, NUM_LOD, FEAT = 16, 256, 8, 2
_growth = math.exp((math.log(MAX_RES) - math.log(BASE_RES)) / (NUM_LOD - 1))
LODS = [int(BASE_RES * _growth ** L) for L in range(NUM_LOD)]   # 16..256
MS = [r - 1 for r in LODS]
N_PTS = 4_000_000
N_CORES = 8
T = 128
PTS_PER_TILE = 128 * T      # 16384
N_TILES = 32
NP_CORE = N_TILES * PTS_PER_TILE   # 524288
MAGIC = 8388608.0

# ---- configuration (override with K3_CFG="GX,GY,treelods,gatherlods") ----
import os
_cfg = os.environ.get("K3_CFG", "32,32,0123456,7")
_p0, _p1, _p2, _p3 = _cfg.split(",")
GX, GY = int(_p0), int(_p1)
TREE_LODS = [int(ch) for ch in _p2]
GATHER_LODS = [int(ch) for ch in _p3] if _p3 else []
# -----------------------

NBLK = GX * GY
B = NBLK // 256             # blocks per tile
PB = 128 // B               # partitions per block band
GB = 8 // B                 # gather groups per block band
CAP = N_PTS and (256 * PTS_PER_TILE) // NBLK   # points per block
assert CAP * NBLK == 256 * PTS_PER_TILE


def _win(m, g):
    w = 0
    for b in range(g):
        w = max(w, ((b + 1) * m) // g - (b * m) // g + 1)
    return w

WX = [_win(m, GX) for m in MS]
WY = [_win(m, GY) for m in MS]
MWIN = [WX[l] * WY[l] for l in range(NUM_LOD)]


def _nlev(M):
    S = (M + 1) // 2
    k = 1
    while S > 1:
        S = S // 2 + (S % 2)
        k += 1
    return k

OFF_M, OFF_C, OFF_R0, OFF_C0, OFF_CL = 0, 8, 16, 24, 32
OFF_M16, OFF_RC16 = 40, 56          # [m]*2 ; [c0 x8, r0 x8]
_off = 72
TREE_OFF = {}
for l in TREE_LODS:
    TREE_OFF[l] = _off
    _off += MWIN[l] * 2
GATH_OFF = {}
for l in GATHER_LODS:
    GATH_OFF[l] = _off
    _off += MWIN[l] * 2
W_MTAB = _off
KOUT = 2 * len(TREE_LODS)
NLEV = {l: (_nlev(MWIN[l]) if MWIN[l] > 1 else 1) for l in TREE_LODS}
MAXLEV = max(NLEV.values())


def _core_blocks(c):
    """List of N_TILES*[B (by,bx) blocks] for core c."""
    qy, hx = c >> 1, c & 1
    ys, xs = GY // 4, GX // 2          # blocks per core region side
    blocks = [(qy * ys + i, hx * xs + j) for i in range(ys) for j in range(xs)]
    # tiles pack B consecutive blocks (row-major within the core region)
    return [blocks[t * B:(t + 1) * B] for t in range(N_TILES)]


def _build_kernel(rep=1, do_tree=True, do_gather=True):
    nc = bacc.Bacc("TRN2", target_bir_lowering=False, debug=False,
                   num_devices=N_CORES)
    pts = nc.dram_tensor("pts", [128, N_TILES, T, 2], F32, kind="ExternalInput")
    mtab = nc.dram_tensor("mtab", [N_TILES * B, W_MTAB], F32,
                          kind="ExternalInput")
    out0 = nc.dram_tensor("out0", [N_TILES * 128, T * KOUT], F32,
                          kind="ExternalOutput")
    strips = {l: nc.dram_tensor(f"strip{l}", [NP_CORE, 2], F32,
                                kind="ExternalOutput") for l in GATHER_LODS}

    with tile.TileContext(nc) as tc:
        with tc.tile_pool(name="mtp", bufs=2) as mtp, \
             tc.tile_pool(name="ptp", bufs=2) as ptp, \
             tc.tile_pool(name="wp", bufs=1) as wp, \
             tc.tile_pool(name="mkp", bufs=1) as mkp, \
             tc.tile_pool(name="vp", bufs=1) as vp, \
             tc.tile_pool(name="op", bufs=2) as op_, \
             tc.tile_pool(name="ip", bufs=2) as ip, \
             tc.tile_pool(name="gp", bufs=2) as gp:

            for _r in range(rep):
                for ti in range(N_TILES):
                    mt = mtp.tile([128, W_MTAB], F32, tag="mt")
                    mpitch = mt[:].ap[0][0]
                    for bi in range(B):
                        dst = bass.AP(mt.tensor, mt[:].offset + bi * PB * mpitch,
                                      [[mpitch, PB], [1, W_MTAB]])
                        nc.sync.dma_start(
                            dst, bass.AP(mtab, (ti * B + bi) * W_MTAB,
                                         [[0, PB], [1, W_MTAB]]))
                    pt = ptp.tile([128, T, 2], F32, tag="pt")
                    nc.sync.dma_start(pt[:], pts.ap()[:, ti])

                    def cvec(off):
                        return mt[:, off:off + 8].unsqueeze(1) \
                                 .to_broadcast([128, T, 8])

                    Cv, clv = cvec(OFF_C), cvec(OFF_CL)

                    A = wp.tile([128, T, 2, 8], F32, tag="A")
                    Bt = wp.tile([128, T, 2, 8], F32, tag="B")
                    FXY = wp.tile([128, T, 2, 8], F32, tag="FXY")
                    LOC = wp.tile([128, T, 8], F32, tag="LOC")

                    xyb = pt[:].unsqueeze(3).to_broadcast([128, T, 2, 8])
                    mv16 = mt[:, OFF_M16:OFF_M16 + 16] \
                        .rearrange("p (a b) -> p a b", b=8) \
                        .unsqueeze(1).to_broadcast([128, T, 2, 8])
                    rc16 = mt[:, OFF_RC16:OFF_RC16 + 16] \
                        .rearrange("p (a b) -> p a b", b=8) \
                        .unsqueeze(1).to_broadcast([128, T, 2, 8])
                    nc.vector.tensor_tensor(out=A[:], in0=xyb, in1=mv16,
                                            op=mybir.AluOpType.mult)
                    nc.vector.tensor_scalar(Bt[:], A[:], MAGIC, -MAGIC,
                                            mybir.AluOpType.add,
                                            mybir.AluOpType.add)
                    nc.vector.tensor_tensor(out=FXY[:], in0=Bt[:], in1=A[:],
                                            op=mybir.AluOpType.is_gt)
                    nc.vector.tensor_tensor(out=Bt[:], in0=Bt[:], in1=FXY[:],
                                            op=mybir.AluOpType.subtract)
                    nc.vector.tensor_tensor(out=FXY[:], in0=Bt[:], in1=rc16,
                                            op=mybir.AluOpType.subtract)
                    nc.vector.tensor_tensor(out=A[:, :, 0, :],
                                            in0=FXY[:, :, 1, :], in1=Cv,
                                            op=mybir.AluOpType.mult)
                    nc.vector.tensor_tensor(out=LOC[:], in0=A[:, :, 0, :],
                                            in1=FXY[:, :, 0, :],
                                            op=mybir.AluOpType.add)
                    nc.vector.tensor_tensor(out=LOC[:], in0=LOC[:], in1=clv,
                                            op=mybir.AluOpType.min)

                    # gather lods first so Pool overlaps the DVE tree
                    for gi, l in enumerate(GATHER_LODS if do_gather else []):
                        M = MWIN[l]
                        tabv = mt[:, GATH_OFF[l]:GATH_OFF[l] + 2 * M] \
                            .rearrange("p (a b) -> p a b", b=2)
                        idx16 = ip.tile([128, T], I16, tag=f"ix{gi}")
                        nc.vector.tensor_copy(out=idx16[:], in_=LOC[:, :, l])
                        gt = gp.tile([128, 2048, 2], F32, tag="gt")
                        nc.gpsimd.ap_gather(gt[:], tabv, idx16[:],
                                            channels=128, num_elems=M,
                                            d=2, num_idxs=2048)
                        gpitch = 2048 * 2
                        srcx = bass.AP(gt.tensor, gt[:].offset,
                                       [[16 * gpitch, 8], [1, 4096]])
                        dstx = bass.AP(strips[l], ti * PTS_PER_TILE * 2,
                                       [[4096, 8], [1, 4096]])
                        nc.gpsimd.dma_start(dstx, srcx)

                    if do_tree:
                        LI = wp.tile([128, T, 8], I16, tag="LI")
                        nc.vector.tensor_copy(out=LI[:], in_=LOC[:])
                        # level-k bit masks, only for lods deep enough
                        mks = {}
                        for k in range(MAXLEV):
                            need = [l for l in TREE_LODS if NLEV[l] > k]
                            lo, hi = min(need), max(need) + 1
                            mk = mkp.tile([128, T, 8], I16, tag=f"mk{k}")
                            nc.vector.tensor_scalar(mk[:, :, lo:hi],
                                                    LI[:, :, lo:hi],
                                                    1 << k, None,
                                                    mybir.AluOpType.bitwise_and)
                            mks[k] = mk
                        ot = op_.tile([128, T, KOUT], F32, tag="ot")
                        Vs, Ss, STs = {}, {}, {}
                        # level 0: ACT copies even cands, DVE predicates odd
                        for l in TREE_LODS:
                            M = MWIN[l]
                            S0 = (M + 1) // 2
                            V = vp.tile([128, T, S0, 2], F32, tag=f"V{l}")
                            Vs[l], Ss[l], STs[l] = V, S0, 1
                            tb = mt[:, TREE_OFF[l]:TREE_OFF[l] + 2 * M] \
                                .rearrange("p (a b) -> p a b", b=2)
                            ev = tb[:, 0::2, :].unsqueeze(1) \
                                .to_broadcast([128, T, S0, 2])
                            nc.scalar.copy(out=V[:], in_=ev)
                        for l in TREE_LODS:
                            M = MWIN[l]
                            no = M // 2
                            if no == 0:
                                continue
                            tb = mt[:, TREE_OFF[l]:TREE_OFF[l] + 2 * M] \
                                .rearrange("p (a b) -> p a b", b=2)
                            od = tb[:, 1::2, :].unsqueeze(1) \
                                .to_broadcast([128, T, no, 2])
                            m0 = mks[0][:, :, l].unsqueeze(2).unsqueeze(3) \
                                .to_broadcast([128, T, no, 2])
                            nc.vector.copy_predicated(
                                Vs[l][:, :, :no, :], m0, od)
                        # in-place levels, interleaved across lods
                        for k in range(1, MAXLEV):
                            for l in TREE_LODS:
                                if Ss[l] <= 1:
                                    continue
                                V = Vs[l]
                                S = Ss[l]
                                st = STs[l]
                                ns = S // 2
                                pstride = V[:].ap[0][0]
                                tstride = V[:].ap[1][0]
                                base_off = V[:].offset
                                mkl = mks[k][:, :, l].unsqueeze(2).unsqueeze(3) \
                                    .to_broadcast([128, T, ns, 2])
                                dstap = bass.AP(V.tensor, base_off,
                                                [[pstride, 128], [tstride, T],
                                                 [4 * st, ns], [1, 2]])
                                srcap = bass.AP(V.tensor, base_off + 2 * st,
                                                [[pstride, 128], [tstride, T],
                                                 [4 * st, ns], [1, 2]])
                                nc.vector.copy_predicated(dstap, mkl, srcap)
                                Ss[l] = ns + (S % 2)
                                STs[l] = st * 2
                        for li, l in enumerate(TREE_LODS):
                            assert Ss[l] == 1, (l, Ss[l])
                            nc.scalar.copy(out=ot[:, :, 2 * li:2 * li + 2],
                                           in_=Vs[l][:, :, 0, :])
                    else:
                        ot = op_.tile([128, T, KOUT], F32, tag="ot")
                        nc.vector.tensor_copy(out=ot[:, :, 0:2], in_=pt[:])

                    dst = bass.AP(out0, ti * 128 * T * KOUT,
                                  [[T * KOUT, 128], [1, T * KOUT]])
                    nc.sync.dma_start(dst, ot[:])

    nc.compile()
    return nc


_NC_CACHE = {}


def _prep_core(pts_all, counts, offs, order, cbs, c):
    """Build in_map for core c. Returns (in_map, src_slots[NP_CORE] int64)."""
    tiles = _core_blocks(c)
    ptile = np.empty((N_TILES, 128, T, 2), np.float32)
    src = np.full((N_TILES, PTS_PER_TILE), -1, np.int64)
    mtab = np.zeros((N_TILES * B, W_MTAB), np.float32)
    for ti, blks in enumerate(tiles):
        for bi, (by, bx) in enumerate(blks):
            b = by * GX + bx
            cnt = min(counts[b], CAP)
            idxs = order[offs[b]:offs[b] + cnt]
            seg = pts_all[idxs]
            if cnt < CAP:
                pad = np.tile(np.array([[(bx + 0.5) / GX, (by + 0.5) / GY]],
                                       np.float32), (CAP - cnt, 1))
                seg = np.concatenate([seg, pad], 0)
            src[ti, bi * CAP:bi * CAP + cnt] = idxs
            # band slots: s = g*2048 + t*16 + q -> partition bi*PB+g*16+q
            ptile[ti, bi * PB:(bi + 1) * PB] = \
                seg.reshape(GB, T, 16, 2).transpose(0, 2, 1, 3) \
                   .reshape(PB, T, 2)
            row = mtab[ti * B + bi]
            for l in range(NUM_LOD):
                m = MS[l]
                row[OFF_M + l] = m
                row[OFF_C + l] = WX[l]
                row[OFF_R0 + l] = (by * m) // GY
                row[OFF_C0 + l] = (bx * m) // GX
                row[OFF_CL + l] = MWIN[l] - 1
                row[OFF_M16 + l] = m
                row[OFF_M16 + 8 + l] = m
                row[OFF_RC16 + l] = (bx * m) // GX
                row[OFF_RC16 + 8 + l] = (by * m) // GY
            for l in range(NUM_LOD):
                m, res = MS[l], LODS[l]
                r0 = (by * m) // GY
                c0 = (bx * m) // GX
                ri = np.minimum(r0 + np.arange(WY[l]), res - 1)
                ci = np.minimum(c0 + np.arange(WX[l]), res - 1)
                vals = cbs[l][(ri[:, None] * res + ci[None, :]).ravel()]
                off0 = TREE_OFF[l] if l in TREE_OFF else GATH_OFF[l]
                row[off0:off0 + 2 * MWIN[l]] = vals.ravel()
    in_map = {"pts": np.ascontiguousarray(ptile.transpose(1, 0, 2, 3)),
              "mtab": mtab}
    return in_map, src.ravel()


def _host_bin(pts):
    x64 = pts[:, 0].astype(np.float64)
    y64 = pts[:, 1].astype(np.float64)
    bx = np.clip(np.floor(x64 * GX).astype(np.int64), 0, GX - 1)
    by = np.clip(np.floor(y64 * GY).astype(np.int64), 0, GY - 1)
    block = by * GX + bx
    order = np.argsort(block, kind="stable")
    counts = np.bincount(block, minlength=NBLK)
    offs = np.concatenate([[0], np.cumsum(counts)])[:-1]
    overflow = []
    for b in np.where(counts > CAP)[0]:
        overflow.extend(order[offs[b] + CAP: offs[b] + counts[b]].tolist())
    return order, counts, offs, overflow


def kernel(pts, cb0, cb1, cb2, cb3, cb4, cb5, cb6, cb7):
    pts = np.ascontiguousarray(np.asarray(pts, dtype=np.float32))
    cbs = [np.ascontiguousarray(np.asarray(cb, dtype=np.float32))
           for cb in (cb0, cb1, cb2, cb3, cb4, cb5, cb6, cb7)]
    assert pts.shape == (N_PTS, 2)

    if "nc" not in _NC_CACHE:
        _NC_CACHE["nc"] = _build_kernel()
    nc = _NC_CACHE["nc"]

    order, counts, offs, overflow_rows = _host_bin(pts)
    in_maps, srcs = [], []
    for c in range(N_CORES):
        m, s = _prep_core(pts, counts, offs, order, cbs, c)
        in_maps.append(m)
        srcs.append(s)

    res = run_bass_kernel_spmd(nc, in_maps, core_ids=list(range(N_CORES)))

    full = np.empty((N_PTS, 16), np.float32)
    for c in range(N_CORES):
        src = srcs[c]
        valid = src >= 0
        sv = src[valid]
        o0 = res.results[c]["out0"].reshape(N_TILES, 8, 16, T, KOUT) \
            .transpose(0, 1, 3, 2, 4).reshape(NP_CORE, KOUT)[valid]
        for li, l in enumerate(TREE_LODS):
            full[sv, l] = o0[:, 2 * li]
            full[sv, 8 + l] = o0[:, 2 * li + 1]
        for l in GATHER_LODS:
            st = res.results[c][f"strip{l}"][valid]
            full[sv, l] = st[:, 0]
            full[sv, 8 + l] = st[:, 1]

    if overflow_rows:
        ov = np.array(overflow_rows, np.int64)
        for l in range(NUM_LOD):
            m, res_l = MS[l], LODS[l]
            fx = np.floor(pts[ov, 0] * np.float32(m)).astype(np.int32)
            fy = np.floor(pts[ov, 1] * np.float32(m)).astype(np.int32)
            v = cbs[l][fx + fy * res_l]
            full[ov, l] = v[:, 0]
            full[ov, 8 + l] = v[:, 1]
    return full
